# revision 3
# baseline (speedup 1.0000x reference)
"""AlignmentModule kernel for 8 TRN2 NeuronCores (self-contained).

Reference computation (per batch):
  hc = conv1d_k1(relu(conv1d_k3(h^T)))^T            # (S, D) text features
  mc = conv1d_k1(relu(conv1d_k3(relu(conv1d_k3(m^T)))))^T  # (T, D) frame feats
  dist = sqrt(max(|hc|^2 + |mc|^2 - 2 hc.mc^T, 0))  # (S, T)
  dist = where(mask[:, None_s? per-s], dist, 0)
  out = log_softmax(-dist, axis=S)

Sharding: data-parallel over batch, 4 batches per core. All compute is done
in (channel, length) = channel-major layout so TensorE contracts over
channels; scores are computed as (t, s) tiles so the softmax over S runs
along the free axis. Output is produced as (B, T, S) and transposed on host.

The mask is folded in additively: a per-s vector g = -1e6 on masked-out
positions is accumulated into the squared distance via a K=1 matmul; the
clamp max(.,0) then zeroes dist exactly at masked positions, reproducing
`where(mask, dist, 0)`.
"""
import sys

import numpy as np

sys.path.insert(0, "/opt/trn_rl_repo")

import concourse.bass as bass  # noqa: E402
import concourse.tile as tile  # noqa: E402
from concourse import bacc, mybir  # noqa: E402
from concourse.bass_utils import run_bass_kernel_spmd  # noqa: E402

F32 = mybir.dt.float32
F32R = mybir.dt.float32r
AF = mybir.ActivationFunctionType
ALU = mybir.AluOpType

B, S, T, D, F = 32, 512, 2048, 256, 80
NCORES = 8
BPC = B // NCORES  # batches per core
NEG_BIG = -1.0e6

_NC_CACHE = {}


def _build_nc():
    nc = bacc.Bacc("TRN2", target_bir_lowering=False, debug=False,
                   num_devices=NCORES)

    # ---- DRAM parameters (per-core shapes) ----
    hTp_d = nc.dram_tensor("hTp", [BPC, D, S + 2], F32R, kind="ExternalInput").ap()
    m3_d = nc.dram_tensor("m3", [BPC, 3 * F, T + 4], F32R, kind="ExternalInput").ap()
    g_d = nc.dram_tensor("gvec", [BPC, S], F32, kind="ExternalInput").ap()
    w1_d = nc.dram_tensor("w1t", [128, 6 * D], F32R, kind="ExternalInput").ap()
    w2_d = nc.dram_tensor("w2t", [128, 2 * D], F32R, kind="ExternalInput").ap()
    f1_d = nc.dram_tensor("f1t", [120, 2 * D], F32R, kind="ExternalInput").ap()
    f2_d = nc.dram_tensor("f2t", [128, 6 * D], F32R, kind="ExternalInput").ap()
    f3_d = nc.dram_tensor("f3t", [128, 2 * D], F32R, kind="ExternalInput").ap()
    bias_d = nc.dram_tensor("biasp", [128, 10], F32, kind="ExternalInput").ap()
    q_d = nc.dram_tensor("qvec", [128, 2], F32R, kind="ExternalInput").ap()
    onesa_d = nc.dram_tensor("onesA", [128, 1], F32R, kind="ExternalInput").ap()
    onesb_d = nc.dram_tensor("onesB", [1, 128], F32R, kind="ExternalInput").ap()
    out_d = nc.dram_tensor("out", [BPC, T, S], F32, kind="ExternalOutput").ap()

    with tile.TileContext(nc) as tc:
        _emit(nc, tc, hTp_d, m3_d, g_d, w1_d, w2_d, f1_d, f2_d, f3_d,
              bias_d, q_d, onesa_d, onesb_d, out_d)
    nc.compile()
    return nc


def _emit(nc, tc, hTp_d, m3_d, g_d, w1_d, w2_d, f1_d, f2_d, f3_d,
          bias_d, q_d, onesa_d, onesb_d, out_d):
    from contextlib import ExitStack
    ctx = ExitStack()
    with ctx:
        wp = ctx.enter_context(tc.tile_pool(name="weights", bufs=1))
        xp = ctx.enter_context(tc.tile_pool(name="acts", bufs=2))
        mcp = ctx.enter_context(tc.tile_pool(name="mc", bufs=2 * 4))
        scp = ctx.enter_context(tc.tile_pool(name="score", bufs=3))
        psA = ctx.enter_context(tc.tile_pool(name="psA", bufs=2, space="PSUM"))
        psB = ctx.enter_context(tc.tile_pool(name="psB", bufs=2, space="PSUM"))
        psR = ctx.enter_context(tc.tile_pool(name="psR", bufs=1, space="PSUM"))
        psM = ctx.enter_context(tc.tile_pool(name="psM", bufs=1, space="PSUM"))

        # ---- load weights/constants once ----
        w1 = wp.tile([128, 6 * D], F32R, tag="w1")
        w2 = wp.tile([128, 2 * D], F32R, tag="w2")
        f1 = wp.tile([120, 2 * D], F32R, tag="f1")
        f2 = wp.tile([128, 6 * D], F32R, tag="f2")
        f3 = wp.tile([128, 2 * D], F32R, tag="f3")
        bias = wp.tile([128, 10], F32, tag="bias")
        qv = wp.tile([128, 2], F32R, tag="qv")
        onesa = wp.tile([128, 1], F32R, tag="onesa")
        onesb = wp.tile([1, 128], F32R, tag="onesb")
        for t_, d_ in ((w1, w1_d), (w2, w2_d), (f1, f1_d), (f2, f2_d),
                       (f3, f3_d), (bias, bias_d), (qv, q_d),
                       (onesa, onesa_d), (onesb, onesb_d)):
            nc.sync.dma_start(t_[:], d_[:])

        for b in range(BPC):
            # ================= TEXT STACK =================
            X = xp.tile([128, 2, S + 2], F32R, tag="X")
            nc.sync.dma_start(
                X[:], hTp_d[b].rearrange("(c p) s -> p c s", p=128))
            gt = xp.tile([1, S], F32, tag="gt")
            nc.sync.dma_start(gt[:], g_d[b][None, :])

            X1 = xp.tile([128, 2, S], F32R, tag="X1")
            for o in range(2):
                ps = psB.tile([128, S], F32, tag="psB")
                first = True
                for c in range(2):
                    for k in range(3):
                        nc.tensor.matmul(
                            ps[:], w1[:, bass.ts(k * 2 + c, D)][:, bass.ts(o, 128)],
                            X[:, c, k:k + S], start=first, stop=(c == 1 and k == 2))
                        first = False
                nc.scalar.activation(X1[:, o, :], ps[:], AF.Relu,
                                     bias=bias[:, 0 + o:1 + o], scale=1.0)

            hcT = xp.tile([128, 2, S], F32R, tag="hcT")
            for o in range(2):
                ps = psB.tile([128, S], F32, tag="psB")
                for c in range(2):
                    nc.tensor.matmul(
                        ps[:], w2[:, bass.ts(c, D)][:, bass.ts(o, 128)],
                        X1[:, c, :], start=(c == 0), stop=(c == 1))
                nc.scalar.activation(hcT[:, o, :], ps[:], AF.Identity,
                                     bias=bias[:, 2 + o:3 + o], scale=1.0)

            # |hc|^2 per s  (+ mask vector) -> hgr (1, S)
            hsq = xp.tile([128, 2, S], F32R, tag="hsq")
            for o in range(2):
                nc.vector.tensor_mul(hsq[:, o, :], hcT[:, o, :], hcT[:, o, :])
            hhp = psR.tile([1, S], F32, tag="hh")
            for o in range(2):
                nc.tensor.matmul(hhp[:], onesa[:], hsq[:, o, :],
                                 start=(o == 0), stop=(o == 1))
            hgr = xp.tile([1, S], F32R, tag="hgr")
            nc.vector.tensor_add(hgr[:], hhp[:], gt[:])

            # ================= FEAT STACK =================
            M2C = []
            mmP = psM.tile([128, 32], F32, tag="mm")
            for tt in range(4):
                M3 = xp.tile([120, 2, 514], F32R, tag="M3")
                nc.sync.dma_start(
                    M3[:],
                    m3_d[b].rearrange("(c p) t -> p c t", p=120)[
                        :, :, tt * 512: tt * 512 + 514])

                X2 = xp.tile([128, 2, 514], F32R, tag="X2")
                for o in range(2):
                    psa = psA.tile([128, 514], F32, tag="psA")
                    for c in range(2):
                        nc.tensor.matmul(
                            psa[:, 0:512],
                            f1[0:120, bass.ts(c, D)][:, bass.ts(o, 128)],
                            M3[:, c, 0:512], start=(c == 0), stop=(c == 1))
                    for c in range(2):
                        nc.tensor.matmul(
                            psa[:, 512:514],
                            f1[0:120, bass.ts(c, D)][:, bass.ts(o, 128)],
                            M3[:, c, 512:514], start=(c == 0), stop=(c == 1))
                    nc.scalar.activation(X2[:, o, :], psa[:], AF.Relu,
                                         bias=bias[:, 4 + o:5 + o], scale=1.0)

                X3 = xp.tile([128, 2, 512], F32R, tag="X3")
                for o in range(2):
                    ps = psB.tile([128, 512], F32, tag="psB")
                    first = True
                    for c in range(2):
                        for k in range(3):
                            nc.tensor.matmul(
                                ps[:], f2[:, bass.ts(k * 2 + c, D)][:, bass.ts(o, 128)],
                                X2[:, c, k:k + 512], start=first,
                                stop=(c == 1 and k == 2))
                            first = False
                    nc.scalar.activation(X3[:, o, :], ps[:], AF.Relu,
                                         bias=bias[:, 6 + o:7 + o], scale=1.0)

                m2c = mcp.tile([128, 2, 512], F32R, tag="m2c")
                for o in range(2):
                    ps = psB.tile([128, 512], F32, tag="psB")
                    for c in range(2):
                        nc.tensor.matmul(
                            ps[:], f3[:, bass.ts(c, D)][:, bass.ts(o, 128)],
                            X3[:, c, :], start=(c == 0), stop=(c == 1))
                    nc.scalar.activation(m2c[:, o, :], ps[:], AF.Identity,
                                         bias=bias[:, 8 + o:9 + o], scale=1.0)
                M2C.append(m2c)

                # |mc|^2 for the 4 t-subtiles of this T-tile
                msq = xp.tile([128, 2, 512], F32R, tag="msq")
                for o in range(2):
                    nc.vector.tensor_mul(msq[:, o, :], m2c[:, o, :], m2c[:, o, :])
                for j in range(4):
                    col = tt * 4 + j
                    for o in range(2):
                        nc.tensor.matmul(
                            mmP[:, 2 * col:2 * col + 2],
                            msq[:, o, bass.ts(j, 128)], qv[:],
                            start=(o == 0), stop=(o == 1))

            mmv = xp.tile([128, 32], F32, tag="mmv")
            nc.vector.tensor_copy(mmv[:], mmP[:])

            # ================= SCORES =================
            for j2 in range(16):
                tt, j = j2 // 4, j2 % 4
                m2c = M2C[tt]
                ps = psB.tile([128, 512], F32, tag="psB")
                for o in range(2):
                    nc.tensor.matmul(ps[:], m2c[:, o, bass.ts(j, 128)],
                                     hcT[:, o, :], start=(o == 0), stop=False)
                nc.tensor.matmul(ps[:], onesb[:], hgr[:], start=False, stop=True)

                sqc = scp.tile([128, 512], F32, tag="sqc")
                nc.vector.tensor_scalar(sqc[:], ps[:], mmv[:, 2 * j2:2 * j2 + 1], 0.0,
                                        ALU.add, ALU.max)
                dist = scp.tile([128, 512], F32, tag="dist")
                nc.scalar.activation(dist[:], sqc[:], AF.Sqrt)
                e = scp.tile([128, 512], F32, tag="e")
                ssum = scp.tile([128, 1], F32, tag="ssum")
                nc.scalar.activation(e[:], dist[:], AF.Exp, scale=-1.0,
                                     accum_out=ssum[:])
                lse = scp.tile([128, 1], F32, tag="lse")
                nc.scalar.activation(lse[:], ssum[:], AF.Ln)
                obuf = scp.tile([128, 512], F32, tag="obuf")
                nc.gpsimd.tensor_scalar(obuf[:], dist[:], lse[:], -1.0,
                                        ALU.add, ALU.mult)
                nc.sync.dma_start(out_d[b, bass.ts(j2, 128), :], obuf[:])


def _prep(h, m, mask, tw1, tb1, tw2, tb2, fw1, fb1, fw2, fb2, fw3, fb3):
    f32 = np.float32
    h = np.asarray(h, f32)
    m = np.asarray(m, f32)
    mask = np.asarray(mask)

    hT = h.transpose(0, 2, 1)                      # (B, D, S)
    hTp = np.zeros((B, D, S + 2), f32)
    hTp[:, :, 1:S + 1] = hT

    mT = m.transpose(0, 2, 1)                      # (B, F, T)
    mTpad = np.zeros((B, F, T + 4), f32)
    mTpad[:, :, 2:T + 2] = mT
    m3 = np.zeros((B, 3 * F, T + 4), f32)
    for k in range(3):
        m3[:, k * F:(k + 1) * F, :T + 4 - k] = mTpad[:, :, k:]

    gvec = np.where(mask, 0.0, NEG_BIG).astype(f32)

    tw1 = np.asarray(tw1, f32); tw2 = np.asarray(tw2, f32)
    fw1 = np.asarray(fw1, f32); fw2 = np.asarray(fw2, f32)
    fw3 = np.asarray(fw3, f32)

    # lhsT layouts: [K(cin) partition, (tap, chunk, cout) free]
    w1t = np.ascontiguousarray(
        tw1.transpose(2, 1, 0).reshape(3, 2, 128, D)
        .transpose(2, 0, 1, 3).reshape(128, 6 * D))
    w2t = np.ascontiguousarray(
        tw2[:, :, 0].T.reshape(2, 128, D).transpose(1, 0, 2).reshape(128, 2 * D))
    W1s = fw1.transpose(2, 1, 0).reshape(3 * F, D)      # (240, 256)
    f1t = np.ascontiguousarray(
        W1s.reshape(2, 120, D).transpose(1, 0, 2).reshape(120, 2 * D))
    f2t = np.ascontiguousarray(
        fw2.transpose(2, 1, 0).reshape(3, 2, 128, D)
        .transpose(2, 0, 1, 3).reshape(128, 6 * D))
    f3t = np.ascontiguousarray(
        (-2.0 * fw3[:, :, 0]).T.reshape(2, 128, D)
        .transpose(1, 0, 2).reshape(128, 2 * D))

    biasp = np.zeros((128, 10), f32)
    for i, bv in enumerate((tb1, tb2, fb1, fb2, -2.0 * np.asarray(fb3, f32))):
        bv = np.asarray(bv, f32).reshape(2, 128).T
        biasp[:, 2 * i:2 * i + 2] = bv
    qvec = np.zeros((128, 2), f32); qvec[:, 0] = 0.25
    onesA = np.ones((128, 1), f32)
    onesB = np.ones((1, 128), f32)

    shared = dict(w1t=w1t, w2t=w2t, f1t=f1t, f2t=f2t, f3t=f3t,
                  biasp=biasp, qvec=qvec, onesA=onesA, onesB=onesB)
    in_maps = []
    for i in range(NCORES):
        sl = slice(i * BPC, (i + 1) * BPC)
        in_maps.append(dict(
            hTp=np.ascontiguousarray(hTp[sl]),
            m3=np.ascontiguousarray(m3[sl]),
            gvec=np.ascontiguousarray(gvec[sl]),
            **shared))
    return in_maps


def get_nc():
    if "nc" not in _NC_CACHE:
        _NC_CACHE["nc"] = _build_nc()
    return _NC_CACHE["nc"]


def run(in_maps, **kw):
    nc = get_nc()
    return run_bass_kernel_spmd(nc, in_maps, core_ids=list(range(NCORES)), **kw)


def kernel(**inputs):
    in_maps = _prep(**inputs)
    res = run(in_maps)
    full = np.concatenate([res.results[i]["out"] for i in range(NCORES)], axis=0)
    return np.ascontiguousarray(full.transpose(0, 2, 1))


# revision 4
# speedup vs baseline: 1.0586x; 1.0586x over previous
"""AlignmentModule kernel for 8 TRN2 NeuronCores (self-contained).

Reference computation (per batch):
  hc = conv1d_k1(relu(conv1d_k3(h^T)))^T            # (S, D) text features
  mc = conv1d_k1(relu(conv1d_k3(relu(conv1d_k3(m^T)))))^T  # (T, D) frame feats
  dist = sqrt(max(|hc|^2 + |mc|^2 - 2 hc.mc^T, 0))  # (S, T)
  dist = where(mask[:, None_s? per-s], dist, 0)
  out = log_softmax(-dist, axis=S)

Sharding: data-parallel over batch, 4 batches per core. All compute is done
in (channel, length) = channel-major layout so TensorE contracts over
channels; scores are computed as (t, s) tiles so the softmax over S runs
along the free axis. Output is produced as (B, T, S) and transposed on host.

The mask is folded in additively: a per-s vector g = -1e6 on masked-out
positions is accumulated into the squared distance via a K=1 matmul; the
clamp max(.,0) then zeroes dist exactly at masked positions, reproducing
`where(mask, dist, 0)`.
"""
import sys

import numpy as np

sys.path.insert(0, "/opt/trn_rl_repo")

import concourse.bass as bass  # noqa: E402
import concourse.tile as tile  # noqa: E402
from concourse import bacc, mybir  # noqa: E402
from concourse.bass_utils import run_bass_kernel_spmd  # noqa: E402

F32 = mybir.dt.float32
F32R = mybir.dt.float32r
AF = mybir.ActivationFunctionType
ALU = mybir.AluOpType

B, S, T, D, F = 32, 512, 2048, 256, 80
NCORES = 8
BPC = B // NCORES  # batches per core
NEG_BIG = -1.0e6

_NC_CACHE = {}


def _build_nc():
    nc = bacc.Bacc("TRN2", target_bir_lowering=False, debug=False,
                   num_devices=NCORES)

    # ---- DRAM parameters (per-core shapes) ----
    hTp_d = nc.dram_tensor("hTp", [BPC, D, S + 2], F32R, kind="ExternalInput").ap()
    m3_d = nc.dram_tensor("m3", [BPC, 3 * F, T + 4], F32R, kind="ExternalInput").ap()
    g_d = nc.dram_tensor("gvec", [BPC, S], F32, kind="ExternalInput").ap()
    w1_d = nc.dram_tensor("w1t", [128, 6 * D], F32R, kind="ExternalInput").ap()
    w2_d = nc.dram_tensor("w2t", [128, 2 * D], F32R, kind="ExternalInput").ap()
    f1_d = nc.dram_tensor("f1t", [120, 2 * D], F32R, kind="ExternalInput").ap()
    f2_d = nc.dram_tensor("f2t", [128, 6 * D], F32R, kind="ExternalInput").ap()
    f3_d = nc.dram_tensor("f3t", [128, 2 * D], F32R, kind="ExternalInput").ap()
    bias_d = nc.dram_tensor("biasp", [128, 10], F32, kind="ExternalInput").ap()
    q_d = nc.dram_tensor("qvec", [128, 2], F32R, kind="ExternalInput").ap()
    onesa_d = nc.dram_tensor("onesA", [128, 1], F32R, kind="ExternalInput").ap()
    onesb_d = nc.dram_tensor("onesB", [1, 128], F32R, kind="ExternalInput").ap()
    out_d = nc.dram_tensor("out", [BPC, T, S], F32, kind="ExternalOutput").ap()

    with tile.TileContext(nc) as tc:
        _emit(nc, tc, hTp_d, m3_d, g_d, w1_d, w2_d, f1_d, f2_d, f3_d,
              bias_d, q_d, onesa_d, onesb_d, out_d)
    nc.compile()
    return nc


def _emit(nc, tc, hTp_d, m3_d, g_d, w1_d, w2_d, f1_d, f2_d, f3_d,
          bias_d, q_d, onesa_d, onesb_d, out_d):
    from contextlib import ExitStack
    ctx = ExitStack()
    with ctx:
        wp = ctx.enter_context(tc.tile_pool(name="weights", bufs=1))
        xp = ctx.enter_context(tc.tile_pool(name="acts", bufs=2))
        mcp = ctx.enter_context(tc.tile_pool(name="mc", bufs=8))
        scp = ctx.enter_context(tc.tile_pool(name="score", bufs=2))
        ps = ctx.enter_context(tc.tile_pool(name="ps", bufs=1, space="PSUM"))

        # ---- load weights/constants once ----
        w1 = wp.tile([128, 6 * D], F32R, tag="w1")
        w2 = wp.tile([128, 2 * D], F32R, tag="w2")
        f1 = wp.tile([120, 2 * D], F32R, tag="f1")
        f2 = wp.tile([128, 6 * D], F32R, tag="f2")
        f3 = wp.tile([128, 2 * D], F32R, tag="f3")
        bias = wp.tile([128, 10], F32, tag="bias")
        qv = wp.tile([128, 2], F32R, tag="qv")
        onesa = wp.tile([128, 1], F32R, tag="onesa")
        onesb = wp.tile([1, 128], F32R, tag="onesb")
        for t_, d_ in ((w1, w1_d), (w2, w2_d), (f1, f1_d), (f2, f2_d),
                       (f3, f3_d), (bias, bias_d), (qv, q_d),
                       (onesa, onesa_d), (onesb, onesb_d)):
            nc.sync.dma_start(t_[:], d_[:])

        for b in range(BPC):
            # ================= TEXT STACK =================
            X = xp.tile([128, 2, S + 2], F32R, tag="X")
            nc.sync.dma_start(
                X[:], hTp_d[b].rearrange("(c p) s -> p c s", p=128))
            gt = xp.tile([1, S], F32, tag="gt")
            nc.sync.dma_start(gt[:], g_d[b][None, :])

            X1 = xp.tile([128, 2, S], F32R, tag="X1")
            for o in range(2):
                p = ps.tile([128, S], F32, tag="p512", bufs=4)
                first = True
                for c in range(2):
                    for k in range(3):
                        nc.tensor.matmul(
                            p[:], w1[:, bass.ts(k * 2 + c, D)][:, bass.ts(o, 128)],
                            X[:, c, k:k + S], start=first, stop=(c == 1 and k == 2))
                        first = False
                nc.scalar.activation(X1[:, o, :], p[:], AF.Relu,
                                     bias=bias[:, 0 + o:1 + o], scale=1.0)

            hcT = xp.tile([128, 2, S], F32R, tag="hcT")
            for o in range(2):
                p = ps.tile([128, S], F32, tag="p512", bufs=4)
                for c in range(2):
                    nc.tensor.matmul(
                        p[:], w2[:, bass.ts(c, D)][:, bass.ts(o, 128)],
                        X1[:, c, :], start=(c == 0), stop=(c == 1))
                nc.scalar.activation(hcT[:, o, :], p[:], AF.Identity,
                                     bias=bias[:, 2 + o:3 + o], scale=1.0)

            # |hc|^2 per s  (+ mask vector) -> hgr (1, S)
            hsq = xp.tile([128, 2, S], F32R, tag="hsq")
            nc.vector.tensor_mul(hsq[:, :, :], hcT[:, :, :], hcT[:, :, :])
            hhp = ps.tile([1, S], F32, tag="row")
            for o in range(2):
                nc.tensor.matmul(hhp[:], onesa[:], hsq[:, o, :],
                                 start=(o == 0), stop=(o == 1))
            hgr = xp.tile([1, S], F32R, tag="hgr")
            nc.vector.tensor_add(hgr[:], hhp[:], gt[:])

            # ================= FEAT STACK =================
            M2C = []
            mmP = ps.tile([128, 32], F32, tag="mm")
            for tt in range(4):
                M3 = xp.tile([120, 2, 514], F32R, tag="M3")
                nc.sync.dma_start(
                    M3[:],
                    m3_d[b].rearrange("(c p) t -> p c t", p=120)[
                        :, :, tt * 512: tt * 512 + 514])

                X2 = xp.tile([128, 2, 514], F32R, tag="X2")
                for o in range(2):
                    pa = ps.tile([128, 514], F32, tag="p514", bufs=1)
                    for c in range(2):
                        nc.tensor.matmul(
                            pa[:, 0:512],
                            f1[0:120, bass.ts(c, D)][:, bass.ts(o, 128)],
                            M3[:, c, 0:512], start=(c == 0), stop=(c == 1))
                    for c in range(2):
                        nc.tensor.matmul(
                            pa[:, 512:514],
                            f1[0:120, bass.ts(c, D)][:, bass.ts(o, 128)],
                            M3[:, c, 512:514], start=(c == 0), stop=(c == 1))
                    nc.scalar.activation(X2[:, o, :], pa[:], AF.Relu,
                                         bias=bias[:, 4 + o:5 + o], scale=1.0)

                X3 = xp.tile([128, 2, 512], F32R, tag="X3")
                for o in range(2):
                    p = ps.tile([128, 512], F32, tag="p512", bufs=4)
                    first = True
                    for c in range(2):
                        for k in range(3):
                            nc.tensor.matmul(
                                p[:], f2[:, bass.ts(k * 2 + c, D)][:, bass.ts(o, 128)],
                                X2[:, c, k:k + 512], start=first,
                                stop=(c == 1 and k == 2))
                            first = False
                    nc.vector.tensor_scalar(X3[:, o, :], p[:],
                                            bias[:, 6 + o:7 + o], 0.0,
                                            ALU.add, ALU.max)

                m2c = mcp.tile([128, 2, 512], F32R, tag="m2c")
                for o in range(2):
                    p = ps.tile([128, 512], F32, tag="p512", bufs=4)
                    for c in range(2):
                        nc.tensor.matmul(
                            p[:], f3[:, bass.ts(c, D)][:, bass.ts(o, 128)],
                            X3[:, c, :], start=(c == 0), stop=(c == 1))
                    nc.vector.tensor_scalar(m2c[:, o, :], p[:],
                                            bias[:, 8 + o:9 + o], None,
                                            ALU.add)
                M2C.append(m2c)

                # |mc|^2 for the 4 t-subtiles of this T-tile
                msq = xp.tile([128, 2, 512], F32R, tag="msq")
                nc.vector.tensor_mul(msq[:, :, :], m2c[:, :, :], m2c[:, :, :])
                for j in range(4):
                    col = tt * 4 + j
                    for o in range(2):
                        nc.tensor.matmul(
                            mmP[:, 2 * col:2 * col + 2],
                            msq[:, o, bass.ts(j, 128)], qv[:],
                            start=(o == 0), stop=(o == 1))

            mmv = xp.tile([128, 32], F32, tag="mmv")
            nc.vector.tensor_copy(mmv[:], mmP[:])

            # ================= SCORES (waves) =================
            sqcs, dists = [], []
            for j2 in range(16):
                tt, j = j2 // 4, j2 % 4
                m2c = M2C[tt]
                p = ps.tile([128, 512], F32, tag="p512", bufs=4)
                for o in range(2):
                    nc.tensor.matmul(p[:], m2c[:, o, bass.ts(j, 128)],
                                     hcT[:, o, :], start=(o == 0), stop=False)
                nc.tensor.matmul(p[:], onesb[:], hgr[:], start=False, stop=True)
                sqc = scp.tile([128, 512], F32, tag="sqc", bufs=16)
                nc.vector.tensor_scalar(sqc[:], p[:], mmv[:, 2 * j2:2 * j2 + 1],
                                        0.0, ALU.add, ALU.max)
                sqcs.append(sqc)

            for j2 in range(16):   # sqrt wave (one table set)
                dist = scp.tile([128, 512], F32, tag="dist", bufs=16)
                nc.scalar.activation(dist[:], sqcs[j2][:], AF.Sqrt)
                dists.append(dist)

            lses = []
            for j2 in range(16):   # exp+ln wave (natural_log_exp set)
                e = scp.tile([128, 512], F32, tag="e", bufs=2)
                ssum = scp.tile([128, 1], F32, tag="ssum", bufs=4)
                nc.scalar.activation(e[:], dists[j2][:], AF.Exp, scale=-1.0,
                                     accum_out=ssum[:])
                lse = scp.tile([128, 1], F32, tag="lse", bufs=16)
                nc.scalar.activation(lse[:], ssum[:], AF.Ln)
                lses.append(lse)

            for j2 in range(16):   # finals on gpsimd + DMA out
                obuf = scp.tile([128, 512], F32, tag="obuf", bufs=4)
                nc.gpsimd.tensor_scalar(obuf[:], dists[j2][:], lses[j2][:],
                                        -1.0, ALU.add, ALU.mult)
                nc.sync.dma_start(out_d[b, bass.ts(j2, 128), :], obuf[:])


def _prep(h, m, mask, tw1, tb1, tw2, tb2, fw1, fb1, fw2, fb2, fw3, fb3):
    f32 = np.float32
    h = np.asarray(h, f32)
    m = np.asarray(m, f32)
    mask = np.asarray(mask)

    hT = h.transpose(0, 2, 1)                      # (B, D, S)
    hTp = np.zeros((B, D, S + 2), f32)
    hTp[:, :, 1:S + 1] = hT

    mT = m.transpose(0, 2, 1)                      # (B, F, T)
    mTpad = np.zeros((B, F, T + 4), f32)
    mTpad[:, :, 2:T + 2] = mT
    m3 = np.zeros((B, 3 * F, T + 4), f32)
    for k in range(3):
        m3[:, k * F:(k + 1) * F, :T + 4 - k] = mTpad[:, :, k:]

    gvec = np.where(mask, 0.0, NEG_BIG).astype(f32)

    tw1 = np.asarray(tw1, f32); tw2 = np.asarray(tw2, f32)
    fw1 = np.asarray(fw1, f32); fw2 = np.asarray(fw2, f32)
    fw3 = np.asarray(fw3, f32)

    # lhsT layouts: [K(cin) partition, (tap, chunk, cout) free]
    w1t = np.ascontiguousarray(
        tw1.transpose(2, 1, 0).reshape(3, 2, 128, D)
        .transpose(2, 0, 1, 3).reshape(128, 6 * D))
    w2t = np.ascontiguousarray(
        tw2[:, :, 0].T.reshape(2, 128, D).transpose(1, 0, 2).reshape(128, 2 * D))
    W1s = fw1.transpose(2, 1, 0).reshape(3 * F, D)      # (240, 256)
    f1t = np.ascontiguousarray(
        W1s.reshape(2, 120, D).transpose(1, 0, 2).reshape(120, 2 * D))
    f2t = np.ascontiguousarray(
        fw2.transpose(2, 1, 0).reshape(3, 2, 128, D)
        .transpose(2, 0, 1, 3).reshape(128, 6 * D))
    f3t = np.ascontiguousarray(
        (-2.0 * fw3[:, :, 0]).T.reshape(2, 128, D)
        .transpose(1, 0, 2).reshape(128, 2 * D))

    biasp = np.zeros((128, 10), f32)
    for i, bv in enumerate((tb1, tb2, fb1, fb2, -2.0 * np.asarray(fb3, f32))):
        bv = np.asarray(bv, f32).reshape(2, 128).T
        biasp[:, 2 * i:2 * i + 2] = bv
    qvec = np.zeros((128, 2), f32); qvec[:, 0] = 0.25
    onesA = np.ones((128, 1), f32)
    onesB = np.ones((1, 128), f32)

    shared = dict(w1t=w1t, w2t=w2t, f1t=f1t, f2t=f2t, f3t=f3t,
                  biasp=biasp, qvec=qvec, onesA=onesA, onesB=onesB)
    in_maps = []
    for i in range(NCORES):
        sl = slice(i * BPC, (i + 1) * BPC)
        in_maps.append(dict(
            hTp=np.ascontiguousarray(hTp[sl]),
            m3=np.ascontiguousarray(m3[sl]),
            gvec=np.ascontiguousarray(gvec[sl]),
            **shared))
    return in_maps


def get_nc():
    if "nc" not in _NC_CACHE:
        _NC_CACHE["nc"] = _build_nc()
    return _NC_CACHE["nc"]


def run(in_maps, **kw):
    nc = get_nc()
    return run_bass_kernel_spmd(nc, in_maps, core_ids=list(range(NCORES)), **kw)


def kernel(**inputs):
    in_maps = _prep(**inputs)
    res = run(in_maps)
    full = np.concatenate([res.results[i]["out"] for i in range(NCORES)], axis=0)
    return np.ascontiguousarray(full.transpose(0, 2, 1))


# revision 5
# speedup vs baseline: 1.6178x; 1.5283x over previous
"""AlignmentModule kernel for 8 TRN2 NeuronCores (self-contained).

Reference computation (per batch):
  hc = conv1d_k1(relu(conv1d_k3(h^T)))^T            # (S, D) text features
  mc = conv1d_k1(relu(conv1d_k3(relu(conv1d_k3(m^T)))))^T  # (T, D) frame feats
  dist = sqrt(max(|hc|^2 + |mc|^2 - 2 hc.mc^T, 0))  # (S, T)
  dist = where(mask[:, None_s? per-s], dist, 0)
  out = log_softmax(-dist, axis=S)

Sharding: data-parallel over batch, 4 batches per core. All compute is done
in (channel, length) = channel-major layout so TensorE contracts over
channels; scores are computed as (t, s) tiles so the softmax over S runs
along the free axis. Output is produced as (B, T, S) and transposed on host.

The mask is folded in additively: a per-s vector g = -1e6 on masked-out
positions is accumulated into the squared distance via a K=1 matmul; the
clamp max(.,0) then zeroes dist exactly at masked positions, reproducing
`where(mask, dist, 0)`.
"""
import sys

import numpy as np

sys.path.insert(0, "/opt/trn_rl_repo")

import concourse.bass as bass  # noqa: E402
import concourse.tile as tile  # noqa: E402
from concourse import bacc, mybir  # noqa: E402
from concourse.bass_utils import run_bass_kernel_spmd  # noqa: E402

F32 = mybir.dt.float32
F32R = mybir.dt.float32r
AF = mybir.ActivationFunctionType
ALU = mybir.AluOpType

B, S, T, D, F = 32, 512, 2048, 256, 80
NCORES = 8
BPC = B // NCORES  # batches per core
NEG_BIG = -1.0e6

_NC_CACHE = {}


def _build_nc():
    nc = bacc.Bacc("TRN2", target_bir_lowering=False, debug=False,
                   num_devices=NCORES)

    # ---- DRAM parameters (per-core shapes) ----
    hTp_d = nc.dram_tensor("hTp", [BPC, D, S + 2], F32R, kind="ExternalInput").ap()
    m3_d = nc.dram_tensor("m3", [BPC, 3 * F, T + 4], F32R, kind="ExternalInput").ap()
    g_d = nc.dram_tensor("gvec", [BPC, S], F32, kind="ExternalInput").ap()
    w1_d = nc.dram_tensor("w1t", [128, 6 * D], F32R, kind="ExternalInput").ap()
    w2_d = nc.dram_tensor("w2t", [128, 2 * D], F32R, kind="ExternalInput").ap()
    f1_d = nc.dram_tensor("f1t", [120, 2 * D], F32R, kind="ExternalInput").ap()
    f2_d = nc.dram_tensor("f2t", [128, 6 * D], F32R, kind="ExternalInput").ap()
    f3_d = nc.dram_tensor("f3t", [128, 2 * D], F32R, kind="ExternalInput").ap()
    bias_d = nc.dram_tensor("biasp", [128, 10], F32, kind="ExternalInput").ap()
    q_d = nc.dram_tensor("qvec", [128, 2], F32R, kind="ExternalInput").ap()
    onesa_d = nc.dram_tensor("onesA", [128, 1], F32R, kind="ExternalInput").ap()
    onesb_d = nc.dram_tensor("onesB", [1, 128], F32R, kind="ExternalInput").ap()
    out_d = nc.dram_tensor("out", [BPC, T, S], F32, kind="ExternalOutput").ap()

    with tile.TileContext(nc) as tc:
        _emit(nc, tc, hTp_d, m3_d, g_d, w1_d, w2_d, f1_d, f2_d, f3_d,
              bias_d, q_d, onesa_d, onesb_d, out_d)
    nc.compile()
    return nc


def _emit(nc, tc, hTp_d, m3_d, g_d, w1_d, w2_d, f1_d, f2_d, f3_d,
          bias_d, q_d, onesa_d, onesb_d, out_d):
    from contextlib import ExitStack
    ctx = ExitStack()
    with ctx:
        wp = ctx.enter_context(tc.tile_pool(name="weights", bufs=1))
        xp = ctx.enter_context(tc.tile_pool(name="acts", bufs=2))
        mcp = ctx.enter_context(tc.tile_pool(name="mc", bufs=8))
        scp = ctx.enter_context(tc.tile_pool(name="score", bufs=2))
        ps = ctx.enter_context(tc.tile_pool(name="ps", bufs=1, space="PSUM"))

        # ---- load weights/constants once ----
        w1 = wp.tile([128, 6 * D], F32R, tag="w1")
        w2 = wp.tile([128, 2 * D], F32R, tag="w2")
        f1 = wp.tile([120, 2 * D], F32R, tag="f1")
        f2 = wp.tile([128, 6 * D], F32R, tag="f2")
        f3 = wp.tile([128, 2 * D], F32R, tag="f3")
        bias = wp.tile([128, 10], F32, tag="bias")
        qv = wp.tile([128, 2], F32R, tag="qv")
        onesa = wp.tile([128, 1], F32R, tag="onesa")
        onesb = wp.tile([1, 128], F32R, tag="onesb")
        for t_, d_ in ((w1, w1_d), (w2, w2_d), (f1, f1_d), (f2, f2_d),
                       (f3, f3_d), (bias, bias_d), (qv, q_d),
                       (onesa, onesa_d), (onesb, onesb_d)):
            nc.sync.dma_start(t_[:], d_[:])

        def text_stack(b):
            X = xp.tile([128, 2, S + 2], F32R, tag="X")
            nc.sync.dma_start(
                X[:], hTp_d[b].rearrange("(c p) s -> p c s", p=128))
            gt = xp.tile([1, S], F32, tag="gt")
            nc.sync.dma_start(gt[:], g_d[b][None, :])

            X1 = xp.tile([128, 2, S], F32R, tag="X1")
            for o in range(2):
                p = ps.tile([128, S], F32, tag="p512", bufs=4)
                first = True
                for c in range(2):
                    for k in range(3):
                        nc.tensor.matmul(
                            p[:], w1[:, bass.ts(k * 2 + c, D)][:, bass.ts(o, 128)],
                            X[:, c, k:k + S], start=first, stop=(c == 1 and k == 2))
                        first = False
                nc.scalar.activation(X1[:, o, :], p[:], AF.Relu,
                                     bias=bias[:, 0 + o:1 + o], scale=1.0)

            hcT = xp.tile([128, 2, S], F32R, tag="hcT")
            for o in range(2):
                p = ps.tile([128, S], F32, tag="p512", bufs=4)
                for c in range(2):
                    nc.tensor.matmul(
                        p[:], w2[:, bass.ts(c, D)][:, bass.ts(o, 128)],
                        X1[:, c, :], start=(c == 0), stop=(c == 1))
                nc.vector.tensor_scalar(hcT[:, o, :], p[:],
                                        bias[:, 2 + o:3 + o], None, ALU.add)

            # |hc|^2 per s  (+ mask vector) -> hgr (1, S)
            hsq = xp.tile([128, 2, S], F32R, tag="hsq")
            nc.gpsimd.tensor_mul(hsq[:, :, :], hcT[:, :, :], hcT[:, :, :])
            hhp = ps.tile([1, S], F32, tag="row")
            for o in range(2):
                nc.tensor.matmul(hhp[:], onesa[:], hsq[:, o, :],
                                 start=(o == 0), stop=(o == 1))
            hgr = xp.tile([1, S], F32R, tag="hgr")
            nc.vector.tensor_add(hgr[:], hhp[:], gt[:])
            return hcT, hgr

        def feat_tiles(b, tts, mmP, M2C):
            for tt in tts:
                M3 = xp.tile([120, 2, 514], F32R, tag="M3")
                nc.sync.dma_start(
                    M3[:],
                    m3_d[b].rearrange("(c p) t -> p c t", p=120)[
                        :, :, tt * 512: tt * 512 + 514])

                X2 = xp.tile([128, 2, 514], F32R, tag="X2")
                for o in range(2):
                    pa = ps.tile([128, 514], F32, tag="p514", bufs=1)
                    for c in range(2):
                        nc.tensor.matmul(
                            pa[:, 0:512],
                            f1[0:120, bass.ts(c, D)][:, bass.ts(o, 128)],
                            M3[:, c, 0:512], start=(c == 0), stop=(c == 1))
                    for c in range(2):
                        nc.tensor.matmul(
                            pa[:, 512:514],
                            f1[0:120, bass.ts(c, D)][:, bass.ts(o, 128)],
                            M3[:, c, 512:514], start=(c == 0), stop=(c == 1))
                    nc.scalar.activation(X2[:, o, :], pa[:], AF.Relu,
                                         bias=bias[:, 4 + o:5 + o], scale=1.0)

                X3 = xp.tile([128, 2, 512], F32R, tag="X3")
                for o in range(2):
                    p = ps.tile([128, 512], F32, tag="p512", bufs=4)
                    first = True
                    for c in range(2):
                        for k in range(3):
                            nc.tensor.matmul(
                                p[:], f2[:, bass.ts(k * 2 + c, D)][:, bass.ts(o, 128)],
                                X2[:, c, k:k + 512], start=first,
                                stop=(c == 1 and k == 2))
                            first = False
                    nc.vector.tensor_scalar(X3[:, o, :], p[:],
                                            bias[:, 6 + o:7 + o], 0.0,
                                            ALU.add, ALU.max)

                m2c = mcp.tile([128, 2, 512], F32R, tag="m2c")
                for o in range(2):
                    p = ps.tile([128, 512], F32, tag="p512", bufs=4)
                    for c in range(2):
                        nc.tensor.matmul(
                            p[:], f3[:, bass.ts(c, D)][:, bass.ts(o, 128)],
                            X3[:, c, :], start=(c == 0), stop=(c == 1))
                    nc.vector.tensor_scalar(m2c[:, o, :], p[:],
                                            bias[:, 8 + o:9 + o], None,
                                            ALU.add)
                M2C.append(m2c)

                # |mc|^2 for the 4 t-subtiles of this T-tile
                msq = xp.tile([128, 2, 512], F32R, tag="msq")
                nc.gpsimd.tensor_mul(msq[:, :, :], m2c[:, :, :], m2c[:, :, :])
                for j in range(4):
                    col = tt * 4 + j
                    for o in range(2):
                        nc.tensor.matmul(
                            mmP[:, 2 * col:2 * col + 2],
                            msq[:, o, bass.ts(j, 128)], qv[:],
                            start=(o == 0), stop=(o == 1))

        # --- score stages for batch b (state dict carries tiles) ---
        def score_mm_clamp(st):
            hcT, hgr, mmv, M2C, b = st["hcT"], st["hgr"], st["mmv"], st["M2C"], st["b"]
            sqcs = []
            for j2 in range(16):
                tt, j = j2 // 4, j2 % 4
                m2c = M2C[tt]
                p = ps.tile([128, 512], F32, tag="p512", bufs=4)
                for o in range(2):
                    nc.tensor.matmul(p[:], m2c[:, o, bass.ts(j, 128)],
                                     hcT[:, o, :], start=(o == 0), stop=False)
                nc.tensor.matmul(p[:], onesb[:], hgr[:], start=False, stop=True)
                sqc = scp.tile([128, 512], F32, tag="sqc", bufs=16)
                nc.vector.tensor_scalar(sqc[:], p[:], mmv[:, 2 * j2:2 * j2 + 1],
                                        0.0, ALU.add, ALU.max)
                sqcs.append(sqc)
            st["sqcs"] = sqcs

        def score_sqrt(st):
            st["dists"] = []
            for j2 in range(16):
                dist = scp.tile([128, 512], F32, tag="dist", bufs=16)
                nc.scalar.activation(dist[:], st["sqcs"][j2][:], AF.Sqrt)
                st["dists"].append(dist)

        def score_exp(st):
            st["ssums"] = []
            for j2 in range(16):
                e = scp.tile([128, 512], F32, tag="e", bufs=2)
                ssum = scp.tile([128, 1], F32, tag="ssum", bufs=16)
                nc.scalar.activation(e[:], st["dists"][j2][:], AF.Exp,
                                     scale=-1.0, accum_out=ssum[:])
                st["ssums"].append(ssum)

        def score_ln_final(st):
            b = st["b"]
            lses = []
            for j2 in range(16):
                lse = scp.tile([128, 1], F32, tag="lse", bufs=16)
                nc.scalar.activation(lse[:], st["ssums"][j2][:], AF.Ln)
                lses.append(lse)
            for j2 in range(16):
                obuf = scp.tile([128, 512], F32, tag="obuf", bufs=4)
                nc.gpsimd.tensor_scalar(obuf[:], st["dists"][j2][:], lses[j2][:],
                                        -1.0, ALU.add, ALU.mult)
                nc.sync.dma_start(out_d[b, bass.ts(j2, 128), :], obuf[:])

        def conv_phase(b):
            hcT, hgr = text_stack(b)
            mmP = ps.tile([128, 32], F32, tag="mm")
            M2C = []
            st = {"b": b, "hcT": hcT, "hgr": hgr, "M2C": M2C, "mmP": mmP}
            return st

        def finish_feat(st):
            mmv = xp.tile([128, 32], F32, tag="mmv")
            nc.vector.tensor_copy(mmv[:], st["mmP"][:])
            st["mmv"] = mmv

        # --- software pipeline over batches ---
        prev = None
        for b in range(BPC):
            st = conv_phase(b)
            feat_tiles(b, [0, 1], st["mmP"], st["M2C"])
            if prev is not None:
                score_sqrt(prev)
            feat_tiles(b, [2, 3], st["mmP"], st["M2C"])
            finish_feat(st)
            if prev is not None:
                score_exp(prev)
                score_ln_final(prev)
            score_mm_clamp(st)
            prev = st
        score_sqrt(prev)
        score_exp(prev)
        score_ln_final(prev)


def _prep(h, m, mask, tw1, tb1, tw2, tb2, fw1, fb1, fw2, fb2, fw3, fb3):
    f32 = np.float32
    h = np.asarray(h, f32)
    m = np.asarray(m, f32)
    mask = np.asarray(mask)

    hT = h.transpose(0, 2, 1)                      # (B, D, S)
    hTp = np.zeros((B, D, S + 2), f32)
    hTp[:, :, 1:S + 1] = hT

    mT = m.transpose(0, 2, 1)                      # (B, F, T)
    mTpad = np.zeros((B, F, T + 4), f32)
    mTpad[:, :, 2:T + 2] = mT
    m3 = np.zeros((B, 3 * F, T + 4), f32)
    for k in range(3):
        m3[:, k * F:(k + 1) * F, :T + 4 - k] = mTpad[:, :, k:]

    gvec = np.where(mask, 0.0, NEG_BIG).astype(f32)

    tw1 = np.asarray(tw1, f32); tw2 = np.asarray(tw2, f32)
    fw1 = np.asarray(fw1, f32); fw2 = np.asarray(fw2, f32)
    fw3 = np.asarray(fw3, f32)

    # lhsT layouts: [K(cin) partition, (tap, chunk, cout) free]
    w1t = np.ascontiguousarray(
        tw1.transpose(2, 1, 0).reshape(3, 2, 128, D)
        .transpose(2, 0, 1, 3).reshape(128, 6 * D))
    w2t = np.ascontiguousarray(
        tw2[:, :, 0].T.reshape(2, 128, D).transpose(1, 0, 2).reshape(128, 2 * D))
    W1s = fw1.transpose(2, 1, 0).reshape(3 * F, D)      # (240, 256)
    f1t = np.ascontiguousarray(
        W1s.reshape(2, 120, D).transpose(1, 0, 2).reshape(120, 2 * D))
    f2t = np.ascontiguousarray(
        fw2.transpose(2, 1, 0).reshape(3, 2, 128, D)
        .transpose(2, 0, 1, 3).reshape(128, 6 * D))
    f3t = np.ascontiguousarray(
        (-2.0 * fw3[:, :, 0]).T.reshape(2, 128, D)
        .transpose(1, 0, 2).reshape(128, 2 * D))

    biasp = np.zeros((128, 10), f32)
    for i, bv in enumerate((tb1, tb2, fb1, fb2, -2.0 * np.asarray(fb3, f32))):
        bv = np.asarray(bv, f32).reshape(2, 128).T
        biasp[:, 2 * i:2 * i + 2] = bv
    qvec = np.zeros((128, 2), f32); qvec[:, 0] = 0.25
    onesA = np.ones((128, 1), f32)
    onesB = np.ones((1, 128), f32)

    shared = dict(w1t=w1t, w2t=w2t, f1t=f1t, f2t=f2t, f3t=f3t,
                  biasp=biasp, qvec=qvec, onesA=onesA, onesB=onesB)
    in_maps = []
    for i in range(NCORES):
        sl = slice(i * BPC, (i + 1) * BPC)
        in_maps.append(dict(
            hTp=np.ascontiguousarray(hTp[sl]),
            m3=np.ascontiguousarray(m3[sl]),
            gvec=np.ascontiguousarray(gvec[sl]),
            **shared))
    return in_maps


def get_nc():
    if "nc" not in _NC_CACHE:
        _NC_CACHE["nc"] = _build_nc()
    return _NC_CACHE["nc"]


def run(in_maps, **kw):
    nc = get_nc()
    return run_bass_kernel_spmd(nc, in_maps, core_ids=list(range(NCORES)), **kw)


def kernel(**inputs):
    in_maps = _prep(**inputs)
    res = run(in_maps)
    full = np.concatenate([res.results[i]["out"] for i in range(NCORES)], axis=0)
    return np.ascontiguousarray(full.transpose(0, 2, 1))
